# revision 4
# baseline (speedup 1.0000x reference)
"""Trainium2 Bass kernel for nn_Decoder_34110630265159.

Computation (full shapes):
  a[n,r,v,h]   = sum_u w1[r,u,v,h] * u[n,r,u,v]
  x[n,ij,r,h]  = relu(sum_v v[n,ij,v] * a[n,r,v,h])
  out[n,ij,x]  = sum_{r,h} w2[x,r,h] * x[n,ij,r,h] * r[n,ij,r]

Sharding: tensor-parallel over the r dimension (64 -> 8 cores x 8).  Each core
reads its r-slice of w1/u/w2/r plus all of v, computes a partial output
(partial over the r-contraction of step 3), and the host sums the partials.

Per-core device plan ("fold" architecture — no r-broadcast DMA):
  step1: K-packed pair matmuls -> T[h, b*8+n]; PE transposes into
         A_n[v, r*128+h] (f16).
  step2: px[rh,ij] = A_n[v,rh].T @ VT_n[v,ij]  (f16, N=512 chunks, PSUM f32)
  relu:  X16 = max(px, 0) -> SBUF f16 (split across ACT/DVE/Pool)
  y:     per (n, ij-chunk): 8 matmuls po[:, r*64:+64] =
         X16_r[h, ij-chunk].T @ W2[h, x-for-r]  (ij on M=128, N=64)
  fold:  YF = po * r_bcast (strided in1 over compact r tile), then a 3-level
         f16 add-tree sums over r -> out[ij, x] per chunk.
"""
import numpy as np

N_CORES = 8
F16 = np.float16

_NC = None


def _build_program():
    import concourse.bass as bass
    import concourse.tile as tile
    from concourse import mybir, bacc

    f32 = mybir.dt.float32
    f16 = mybir.dt.float16
    AF = mybir.ActivationFunctionType
    ALU = mybir.AluOpType

    nc = bacc.Bacc("TRN2", target_bir_lowering=False, debug=False,
                   num_devices=N_CORES, num_swdge_queues=4)

    w1h = nc.declare_dram_parameter("w1h", [128, 32768], f16, isOutput=False)
    uh = nc.declare_dram_parameter("uh", [128, 4096], f16, isOutput=False)
    vth = nc.declare_dram_parameter("vth", [4, 128, 1024], f16, isOutput=False)
    rtc = nc.declare_dram_parameter("rtc", [128, 512], f16, isOutput=False)
    w2t = nc.declare_dram_parameter("w2t", [128, 512], f16, isOutput=False)
    idm = nc.declare_dram_parameter("idm", [128, 128], f16, isOutput=False)
    outp = nc.declare_dram_parameter("outp", [8, 128, 512], f16, isOutput=True)

    with tile.TileContext(nc) as tc:
        with tc.tile_pool(name="consts", bufs=1) as consts, \
             tc.tile_pool(name="tw", bufs=1) as tw, \
             tc.tile_pool(name="apool", bufs=1) as apool, \
             tc.tile_pool(name="x16p", bufs=18) as x16p, \
             tc.tile_pool(name="yfp", bufs=3) as yfp, \
             tc.tile_pool(name="s1p", bufs=3) as s1p, \
             tc.tile_pool(name="s2p", bufs=3) as s2p, \
             tc.tile_pool(name="obp", bufs=2) as obp:

            # ---- input DMAs, ordered by when compute needs them ----
            us = consts.tile([128, 4096], f16, name="us")
            nc.sync.dma_start(out=us[:, :2048], in_=uh[:, :2048])
            w1s = [consts.tile([128, 4096], f16, name=f"w1s{i}", tag=f"w1s{i}")
                   for i in range(8)]
            for i in range(4):
                eng = nc.scalar if i % 2 == 0 else nc.sync
                eng.dma_start(out=w1s[i][:], in_=w1h[:, i * 4096:(i + 1) * 4096])
            nc.sync.dma_start(out=us[:, 2048:], in_=uh[:, 2048:])
            vtl = [consts.tile([128, 1024], f16, name=f"vtl{p}", tag=f"vtl{p}")
                   for p in range(4)]
            nc.sync.dma_start(out=vtl[0][:], in_=vth[0, :, :])
            ids = consts.tile([128, 128], f16, name="ids")
            nc.scalar.dma_start(out=ids[:], in_=idm[:, :])
            for i in range(4, 8):
                eng = nc.scalar if i % 2 == 0 else nc.sync
                eng.dma_start(out=w1s[i][:], in_=w1h[:, i * 4096:(i + 1) * 4096])
            for p in range(1, 4):
                eng = nc.sync if p % 2 == 0 else nc.scalar
                eng.dma_start(out=vtl[p][:], in_=vth[p, :, :])
            w2s = consts.tile([128, 512], f16, name="w2s")
            nc.sync.dma_start(out=w2s[:], in_=w2t[:, :])
            rts = consts.tile([128, 512], f16, name="rts")
            nc.scalar.dma_start(out=rts[:], in_=rtc[:, :])

            tsb = tw.tile([128, 4096], f16, name="tsb")
            asb = [apool.tile([128, 1024], f16, name=f"asb{p}", tag=f"asb{p}")
                   for p in range(4)]
            tsbv = tsb[:].rearrange("p (rl v n) -> p rl v n", rl=8, v=64, n=8)

            # PSUM budget (8 banks): psA (2) for step1+transposes closes
            # before psx (6) + pso (2) open for the main loop.
            with tc.tile_pool(name="psA", bufs=2, space="PSUM") as psA:

                def step1_round(rnd):
                    pb = psA.tile([128, 512], f32, name="pb", tag="pb")
                    for s in range(32):
                        p = rnd * 32 + s
                        nc.tensor.matmul(
                            pb[:, s * 16:(s + 1) * 16],
                            lhsT=w1s[rnd][:, s * 128:(s + 1) * 128],
                            rhs=us[:, p * 16:(p + 1) * 16],
                            start=True, stop=True,
                        )
                    dst = tsb[:, rnd * 512:(rnd + 1) * 512]
                    if rnd % 2 == 0:
                        nc.vector.tensor_copy(dst, pb[:])
                    else:
                        nc.scalar.activation(dst, pb[:], AF.Copy)

                tk = 0

                def transposes(nn, g):
                    nonlocal tk
                    pt = psA.tile([64, 512], f16, name="pt", tag="pb")
                    for j in range(4):
                        rl = g * 4 + j
                        nc.tensor.transpose(
                            pt[:, j * 128:(j + 1) * 128],
                            tsbv[:, rl, :, nn], ids[:])
                    dst = asb[nn // 2][(nn % 2) * 64:(nn % 2) * 64 + 64,
                                       g * 512:(g + 1) * 512]
                    if tk % 2 == 0:
                        nc.vector.tensor_copy(dst, pt[:])
                    else:
                        nc.scalar.activation(dst, pt[:], AF.Copy)
                    tk += 1

                for rnd in range(4):
                    step1_round(rnd)
                for nn in range(8):
                    transposes(nn, 0)
                for rnd in range(4, 8):
                    step1_round(rnd)
                for nn in range(8):
                    transposes(nn, 1)

            # ---- main loop over n ----
            psx_cm = tc.tile_pool(name="psx", bufs=3, space="PSUM")
            psx = psx_cm.__enter__()
            pso_cm = tc.tile_pool(name="pso", bufs=2, space="PSUM")
            pso = pso_cm.__enter__()

            xts = {}
            obts = {}

            def step2_relu(nn, t):
                pp, half = nn // 2, nn % 2
                lo, hi = half * 64, half * 64 + 64
                px = psx.tile([128, 1024], f32, name="px", tag="px")
                for c in range(2):
                    nc.tensor.matmul(
                        px[:, c * 512:(c + 1) * 512],
                        lhsT=asb[pp][lo:hi, t * 128:(t + 1) * 128],
                        rhs=vtl[pp][lo:hi, c * 512:(c + 1) * 512],
                        start=True, stop=True,
                        tile_position=(lo, 0),
                    )
                x16 = x16p.tile([128, 1024], f16, name="x16", tag="x16")
                if t < 7:
                    nc.scalar.activation(x16[:], px[:], AF.Relu)
                else:
                    nc.vector.tensor_scalar_max(out=x16[:], in0=px[:],
                                                scalar1=0.0)
                xts[(nn, t)] = x16

            def yfold(m, c):
                if c == 0:
                    obts[m] = obp.tile([128, 512], f16, name="obt", tag="obt")
                po = pso.tile([128, 512], f32, name="po", tag="po")
                for r in range(8):
                    nc.tensor.matmul(
                        po[:, r * 64:(r + 1) * 64],
                        lhsT=xts[(m, r)][:, c * 128:(c + 1) * 128],
                        rhs=w2s[:, r * 64:(r + 1) * 64],
                        start=True, stop=True,
                        skip_group_check=True,
                    )
                # fold r in: YF[p, r, x] = po[p, r, x] * rbc[p, r] (bcast x)
                base = (m * 8 + c) * 8
                rr = rts[:, base:base + 8]
                bc = bass.AP(tensor=rr.tensor, offset=rr.offset,
                             ap=[list(rr.ap[0])] + [[1, 8], [0, 64]])
                yf = yfp.tile([128, 512], f16, name="yf", tag="yf")
                po_v = po[:].rearrange("p (r x) -> p r x", r=8)
                yf_v = yf[:].rearrange("p (r x) -> p r x", r=8)
                nc.vector.tensor_tensor(out=yf_v, in0=po_v, in1=bc,
                                        op=ALU.mult)
                # 3-level f16 add tree over r (r-major halves); the SBUF-only
                # levels go to GpSimd (it cannot touch PSUM).
                s1 = s1p.tile([128, 256], f16, name="s1", tag="s1")
                nc.gpsimd.tensor_tensor(out=s1[:], in0=yf[:, :256],
                                        in1=yf[:, 256:], op=ALU.add)
                s2 = s2p.tile([128, 128], f16, name="s2", tag="s2")
                nc.gpsimd.tensor_tensor(out=s2[:], in0=s1[:, :128],
                                        in1=s1[:, 128:], op=ALU.add)
                nc.vector.tensor_tensor(out=obts[m][:, c * 64:(c + 1) * 64],
                                        in0=s2[:, :64], in1=s2[:, 64:],
                                        op=ALU.add)
                if c == 7:
                    eng = nc.sync if m % 2 == 0 else nc.scalar
                    eng.dma_start(out=outp[m, :, :], in_=obts[m][:])
                    for r in range(8):
                        del xts[(m, r)]

            for nn in range(8):
                for t in range(8):
                    step2_relu(nn, t)
                    if nn >= 1:
                        yfold(nn - 1, t)
            for c in range(8):
                yfold(7, c)

            pso_cm.__exit__(None, None, None)
            psx_cm.__exit__(None, None, None)

    nc.finalize()
    return nc


def _host_pack(core, r, u, v, w1, w2):
    rs = slice(8 * core, 8 * core + 8)

    w1c = w1[rs]  # [8,64,64,128] (rl,u,v,h)
    t = w1c.transpose(0, 2, 1, 3).reshape(512, 64, 128)
    t = t.reshape(256, 2, 64, 128).transpose(1, 2, 0, 3)
    w1h = np.ascontiguousarray(t.reshape(128, 256 * 128)).astype(F16)

    uc = u[:, rs]  # [n, rl, u, v]
    ut = uc.transpose(1, 3, 2, 0).reshape(512, 64, 8).reshape(256, 2, 64, 8)
    usb4 = np.zeros((2, 64, 256, 2, 8), dtype=np.float32)
    usb4[0, :, :, 0, :] = ut[:, 0].transpose(1, 0, 2)
    usb4[1, :, :, 1, :] = ut[:, 1].transpose(1, 0, 2)
    uh = usb4.reshape(128, 4096).astype(F16)

    vth = np.ascontiguousarray(
        v.reshape(8, 1024, 64).transpose(0, 2, 1).reshape(4, 128, 1024)
    ).astype(F16)

    rc = r.reshape(8, 1024, 64)[:, :, rs]  # [n, ij, rl]
    rc = rc.reshape(8, 8, 128, 8)  # [n, chunk, p, rl]
    rtc = np.ascontiguousarray(
        rc.transpose(2, 0, 1, 3).reshape(128, 512)
    ).astype(F16)

    w2t = np.ascontiguousarray(
        w2[:, rs, :].transpose(2, 1, 0).reshape(128, 512)
    ).astype(F16)

    idm = np.eye(128, dtype=np.float32).astype(F16)

    return {"w1h": w1h, "uh": uh, "vth": vth, "rtc": rtc, "w2t": w2t,
            "idm": idm}


def kernel(r, u, v, w1, w2):
    global _NC
    from concourse.bass_utils import run_bass_kernel_spmd

    r = np.asarray(r, dtype=np.float32)
    u = np.asarray(u, dtype=np.float32)
    v = np.asarray(v, dtype=np.float32)
    w1 = np.asarray(w1, dtype=np.float32)
    w2 = np.asarray(w2, dtype=np.float32)

    if _NC is None:
        _NC = _build_program()

    in_maps = [_host_pack(c, r, u, v, w1, w2) for c in range(N_CORES)]
    res = run_bass_kernel_spmd(_NC, in_maps, list(range(N_CORES)))

    acc = np.zeros((8, 1024, 64), dtype=np.float32)
    for c in range(N_CORES):
        o = res.results[c]["outp"].astype(np.float32)  # [8, 128, 512]
        acc += o.reshape(8, 128, 8, 64).transpose(0, 2, 1, 3).reshape(
            8, 1024, 64)
    out = acc.reshape(8, 32, 32, 64)
    return np.ascontiguousarray(out).astype(np.float32)


# revision 17
# speedup vs baseline: 1.2774x; 1.2774x over previous
"""Trainium2 Bass kernel for nn_Decoder_34110630265159.

Computation (full shapes):
  a[n,r,v,h]   = sum_u w1[r,u,v,h] * u[n,r,u,v]
  x[n,ij,r,h]  = relu(sum_v v[n,ij,v] * a[n,r,v,h])
  out[n,ij,x]  = sum_{r,h} w2[x,r,h] * x[n,ij,r,h] * r[n,ij,r]

Sharding: tensor-parallel over the r dimension (64 -> 8 cores x 8).  Each core
reads its r-slice of w1/u/w2/r plus all of v, computes a partial output
(partial over the r-contraction of step 3), and the host sums the partials.

Per-core device plan:
  step1: K-packed pair matmuls: two (r,v) batches share one K=128 matmul
         -> T[h, b*8+n]
  a-transpose: PE transposes T blocks into A_n[v, r*128+h] (fp16)
  step2: x[rh, ij] = A_n[v,rh].T @ VT_n[v,ij]   (fp16, N=512 chunks)
  relu*r: XR = relu(x) * r[n,ij,r]  via DVE scalar_tensor_tensor or
         ACT relu + DVE multiply; the r broadcast tile comes from a
         partition-stride-0 DMA straight out of HBM.
  step3 (ij-on-M): per (n, ij-chunk): po[ij,x] += XR_r[h,chunk].T @
         W2[h, x-for-r], accumulated over the 8 local r (M=128 ij, N=64).
"""
import os
import numpy as np

N_CORES = 8
F16 = np.float16

_NC = None


def _build_program():
    import concourse.bass as bass
    import concourse.tile as tile
    from concourse import mybir, bacc
    from concourse.tile import add_dep_helper

    f32 = mybir.dt.float32
    f16 = mybir.dt.float16
    AF = mybir.ActivationFunctionType
    ALU = mybir.AluOpType

    nc = bacc.Bacc("TRN2", target_bir_lowering=False, debug=False,
                   num_devices=N_CORES, num_swdge_queues=4)

    w1h = nc.declare_dram_parameter("w1h", [128, 32768], f16, isOutput=False)
    uh = nc.declare_dram_parameter("uh", [128, 4096], f16, isOutput=False)
    vth = nc.declare_dram_parameter("vth", [4, 128, 1024], f16, isOutput=False)
    rth = nc.declare_dram_parameter("rth", [8, 8192], f16, isOutput=False)
    w2t = nc.declare_dram_parameter("w2t", [128, 512], f16, isOutput=False)
    idm = nc.declare_dram_parameter("idm", [128, 128], f16, isOutput=False)
    outp = nc.declare_dram_parameter("outp", [8, 128, 512], f16, isOutput=True)

    STT_MOD = int(os.environ.get("STT_MOD", "4"))
    # quads (nn, t-half) whose r-broadcast comes from a PE ones-matmul
    # instead of the HBM replication DMA (relieves the DMA bottleneck).
    N_PEQ = int(os.environ.get("N_PEQ", "0"))
    PE_QUADS = {(k * 5 + 2) % 16 for k in range(N_PEQ)}

    with tile.TileContext(nc) as tc:
        with tc.tile_pool(name="consts", bufs=1) as consts, \
             tc.tile_pool(name="tw", bufs=1) as tw, \
             tc.tile_pool(name="apool", bufs=1) as apool, \
             tc.tile_pool(name="xrp", bufs=2) as xrp, \
             tc.tile_pool(name="rbp", bufs=6) as rbp, \
             tc.tile_pool(name="xtp", bufs=3) as xtp, \
             tc.tile_pool(name="obp", bufs=2) as obp:

            # ---- input DMAs, ordered by when compute needs them ----
            us = consts.tile([128, 4096], f16, name="us")
            nc.sync.dma_start(out=us[:, :2048], in_=uh[:, :2048])
            w1s = [consts.tile([128, 4096], f16, name=f"w1s{i}", tag=f"w1s{i}")
                   for i in range(8)]
            w1_dmas = []
            for i in range(4):
                eng = nc.scalar if i % 2 == 0 else nc.sync
                w1_dmas.append(eng.dma_start(
                    out=w1s[i][:], in_=w1h[:, i * 4096:(i + 1) * 4096]))
            nc.sync.dma_start(out=us[:, 2048:], in_=uh[:, 2048:])
            vtl = [consts.tile([128, 1024], f16, name=f"vtl{p}", tag=f"vtl{p}")
                   for p in range(4)]
            nc.sync.dma_start(out=vtl[0][:], in_=vth[0, :, :])
            ids = consts.tile([128, 128], f16, name="ids")
            nc.scalar.dma_start(out=ids[:], in_=idm[:, :])
            for i in range(4, 8):
                eng = nc.scalar if i % 2 == 0 else nc.sync
                w1_dmas.append(eng.dma_start(
                    out=w1s[i][:], in_=w1h[:, i * 4096:(i + 1) * 4096]))
            for p in range(1, 4):
                eng = nc.sync if p % 2 == 0 else nc.scalar
                eng.dma_start(out=vtl[p][:], in_=vth[p, :, :])
            w2s = consts.tile([128, 512], f16, name="w2s")
            nc.sync.dma_start(out=w2s[:], in_=w2t[:, :])

            # r broadcast tiles: partition-stride-0 DMA straight from HBM.
            # The first few are pinned behind the w1 input stream so the big
            # replication writes don't starve step 1's weight chunks.
            rbts = []

            def emit_rb(k, pool):
                nn, hh = k // 2, k % 2
                rbt = pool.tile([128, 4096], f16, name="rbt", tag="rbt")
                rrow = rth[nn, hh * 4096:(hh + 1) * 4096]
                bc = bass.AP(tensor=rrow.tensor, offset=rrow.offset,
                             ap=[[0, 128]] + [list(d) for d in rrow.ap])
                rb_dma = nc.gpsimd.dma_start(out=rbt[:], in_=bc)
                if k < 5:
                    add_dep_helper(rb_dma.ins, w1_dmas[7].ins,
                                   reason="rb prefetch after w1 input stream")
                rbts.append(rbt)
                return rb_dma

            for k in range(16):          # (n, t-half) granularity, 1 MB each
                emit_rb(k, rbp)

            tsb = tw.tile([128, 4096], f16, name="tsb")
            asb = [apool.tile([128, 1024], f16, name=f"asb{p}", tag=f"asb{p}")
                   for p in range(4)]
            tsbv = tsb[:].rearrange("p (rl v n) -> p rl v n", rl=8, v=64, n=8)

            # PSUM budget (8 banks), LIFO-nested: psx (6 banks) opens first
            # and spans everything; psA (2 banks) covers step1+transposes and
            # closes before pso (1 bank x 2) opens for the step-3 accumulators.
            psx = None
            xrts = {}
            donelists = {}

            def tile_work(nn, t):
                if nn not in xrts:
                    xrts[nn] = xrp.tile([128, 8192], f16, name="xrt", tag="xrt")
                    donelists[nn] = []
                pp, half = nn // 2, nn % 2
                lo, hi = half * 64, half * 64 + 64
                m = nn * 8 + t
                use_stt = (m % STT_MOD == 0)
                px = psx.tile([128, 1024], f32, name="px", tag="px")
                for c in range(2):
                    nc.tensor.matmul(
                        px[:, c * 512:(c + 1) * 512],
                        lhsT=asb[pp][lo:hi, t * 128:(t + 1) * 128],
                        rhs=vtl[pp][lo:hi, c * 512:(c + 1) * 512],
                        start=True, stop=True,
                        tile_position=(lo, 0),
                    )
                xslice = xrts[nn][:, t * 1024:(t + 1) * 1024]
                rbt = rbts[nn * 2 + t // 4]
                rbs = rbt[:, (t % 4) * 1024:(t % 4 + 1) * 1024]
                if use_stt:
                    nc.vector.scalar_tensor_tensor(
                        out=xslice, in0=px[:], scalar=0.0, in1=rbs,
                        op0=ALU.max, op1=ALU.mult,
                    )
                else:
                    xtmp = xtp.tile([128, 1024], f16, name="xtmp")
                    nc.scalar.activation(xtmp[:], px[:], AF.Relu)
                    nc.vector.tensor_tensor(
                        out=xslice, in0=xtmp[:], in1=rbs, op=ALU.mult,
                    )
                donelists[nn].append(t)

            with tc.tile_pool(name="psA", bufs=2, space="PSUM") as psA:

                def step1_round(rnd):
                    pb = psA.tile([128, 512], f32, name="pb", tag="pb")
                    for s in range(32):
                        p = rnd * 32 + s
                        nc.tensor.matmul(
                            pb[:, s * 16:(s + 1) * 16],
                            lhsT=w1s[rnd][:, s * 128:(s + 1) * 128],
                            rhs=us[:, p * 16:(p + 1) * 16],
                            start=True, stop=True,
                        )
                    dst = tsb[:, rnd * 512:(rnd + 1) * 512]
                    if rnd % 2 == 0:
                        nc.vector.tensor_copy(dst, pb[:])
                    else:
                        nc.scalar.activation(dst, pb[:], AF.Copy)

                tk = 0

                def transposes(nn, g):
                    nonlocal tk
                    pt = psA.tile([64, 512], f16, name="pt", tag="pb")
                    for j in range(4):
                        rl = g * 4 + j
                        nc.tensor.transpose(
                            pt[:, j * 128:(j + 1) * 128],
                            tsbv[:, rl, :, nn], ids[:])
                    dst = asb[nn // 2][(nn % 2) * 64:(nn % 2) * 64 + 64,
                                      g * 512:(g + 1) * 512]
                    if tk % 2 == 0:
                        nc.vector.tensor_copy(dst, pt[:])
                    else:
                        nc.scalar.activation(dst, pt[:], AF.Copy)
                    tk += 1

                for rnd in range(4):
                    step1_round(rnd)
                for nn in range(8):
                    transposes(nn, 0)
                for rnd in range(4, 8):
                    step1_round(rnd)
                for nn in range(8):
                    transposes(nn, 1)

            # ---- main loop over n ----
            # step3 (ij-on-M): for chunk c, the 8 local r's accumulate into
            # po[:, c*64:(c+1)*64] (M=128 ij, N=64 x) — half the PE columns
            # of the x-on-M form.  A chunk needs all 8 r tiles of its n, so
            # step3 for n-1 interleaves with step2/relu of n on the in-order
            # PE queue.
            psx_cm = tc.tile_pool(name="psx", bufs=3, space="PSUM")
            psx = psx_cm.__enter__()
            with tc.tile_pool(name="pso", bufs=2, space="PSUM") as pso:
                pos = {}

                def step3_chunk(m, c):
                    if c == 0:
                        pos[m] = pso.tile([128, 512], f32, name="po",
                                          tag="po")
                    po = pos[m]
                    for r in range(8):
                        nc.tensor.matmul(
                            po[:, c * 64:(c + 1) * 64],
                            lhsT=xrts[m][:, r * 1024 + c * 128:
                                         r * 1024 + (c + 1) * 128],
                            rhs=w2s[:, r * 64:(r + 1) * 64],
                            start=(r == 0), stop=(r == 7),
                            skip_group_check=True,
                        )
                    if c == 7:
                        obt = obp.tile([128, 512], f16, name="obt")
                        nc.scalar.activation(obt[:], po[:], AF.Copy)
                        eng = nc.sync if m % 2 == 0 else nc.scalar
                        eng.dma_start(out=outp[m, :, :], in_=obt[:])
                        del xrts[m]

                for nn in range(8):
                    for t in range(8):
                        tile_work(nn, t)
                        if nn >= 1:
                            step3_chunk(nn - 1, t)
                for c in range(8):
                    step3_chunk(7, c)
            psx_cm.__exit__(None, None, None)

    nc.finalize()
    return nc


def _host_pack(core, r, u, v, w1, w2):
    rs = slice(8 * core, 8 * core + 8)

    w1c = w1[rs]  # [8,64,64,128] (rl,u,v,h)
    t = w1c.transpose(0, 2, 1, 3).reshape(512, 64, 128)
    t = t.reshape(256, 2, 64, 128).transpose(1, 2, 0, 3)
    w1h = np.ascontiguousarray(t.reshape(128, 256 * 128)).astype(F16)

    uc = u[:, rs]  # [n, rl, u, v]
    ut = uc.transpose(1, 3, 2, 0).reshape(512, 64, 8).reshape(256, 2, 64, 8)
    usb4 = np.zeros((2, 64, 256, 2, 8), dtype=np.float32)
    usb4[0, :, :, 0, :] = ut[:, 0].transpose(1, 0, 2)
    usb4[1, :, :, 1, :] = ut[:, 1].transpose(1, 0, 2)
    uh = usb4.reshape(128, 4096).astype(F16)

    vth = np.ascontiguousarray(
        v.reshape(8, 1024, 64).transpose(0, 2, 1).reshape(4, 128, 1024)
    ).astype(F16)

    rc = r.reshape(8, 1024, 64)[:, :, rs]  # [n, ij, rl]
    rth = np.ascontiguousarray(rc.transpose(0, 2, 1).reshape(8, 8192)).astype(F16)

    w2t = np.ascontiguousarray(
        w2[:, rs, :].transpose(2, 1, 0).reshape(128, 512)
    ).astype(F16)

    idm = np.eye(128, dtype=np.float32).astype(F16)

    return {"w1h": w1h, "uh": uh, "vth": vth, "rth": rth, "w2t": w2t, "idm": idm}


def kernel(r, u, v, w1, w2):
    global _NC
    from concourse.bass_utils import run_bass_kernel_spmd

    r = np.asarray(r, dtype=np.float32)
    u = np.asarray(u, dtype=np.float32)
    v = np.asarray(v, dtype=np.float32)
    w1 = np.asarray(w1, dtype=np.float32)
    w2 = np.asarray(w2, dtype=np.float32)

    if _NC is None:
        _NC = _build_program()

    in_maps = [_host_pack(c, r, u, v, w1, w2) for c in range(N_CORES)]
    res = run_bass_kernel_spmd(_NC, in_maps, list(range(N_CORES)))

    acc = np.zeros((8, 1024, 64), dtype=np.float32)
    for c in range(N_CORES):
        o = res.results[c]["outp"].astype(np.float32)  # [8, 128, 512]
        acc += o.reshape(8, 128, 8, 64).transpose(0, 2, 1, 3).reshape(
            8, 1024, 64)
    out = acc.reshape(8, 32, 32, 64)
    return np.ascontiguousarray(out).astype(np.float32)
